# revision 14
# baseline (speedup 1.0000x reference)
"""Trainium2 Bass kernel for BasicNCA (noAddUp): 2 steps of
perceive(Sobel) -> linear MLP (collapsed, no activation between layers)
-> masked sigmoid update, data-parallel over batch across 8 NeuronCores.

Layout per core (batch element): x[256,256,32] -> SBUF [128, 66, 260] bf16
  partition p = 32*g + c   (g = row-group of 64 image rows, c = channel)
  buffer row br = image row (64g + br - 1); br=0/65 are halo rows
  col j: image col = j - 2 for j in [2, 258); j in {0,1,258,259} are zero pads

Math (exact collapse of the reference):
  W_eff = fc1_w @ fc0_w [32,96], b_eff = fc1_w @ fc0_b
  d = Wc*x + (Wdx/8)*(tA@-1 + 2 tA@0 + tA@+1) + (Wdy/8)*(tB@+1 - tB@-1) + b_eff
  where tA(r) = x(r+1)-x(r-1), tB(r) = x(r-1)+2x(r)+x(r+1)   (vertical)
  s(r) = x(r)+x(r+1);  tA = s(r)-s(r-1);  tB = s(r)+s(r-1)
  x_new = sigmoid(x + d*mask) on channels 3..31; channels 0..2 pass through.
"""

import sys

import numpy as np

for _p in ("/opt/trn_rl_repo", "/root/.axon_site/_ro/trn_rl_repo"):
    if _p not in sys.path:
        sys.path.append(_p)

import ml_dtypes  # noqa: E402

import concourse.bass as bass  # noqa: E402
import concourse.tile as tile  # noqa: E402
from concourse import bacc, mybir  # noqa: E402
from concourse.bass_utils import run_bass_kernel_spmd  # noqa: E402

BF16 = ml_dtypes.bfloat16
F32 = mybir.dt.float32
BF = mybir.dt.bfloat16

N_CORES = 8
ROWS = 256
COLS = 256
CH = 32
GROUPS = 4
GROW = ROWS // GROUPS  # 64 rows per group
RS = 260  # padded row stride: [pad, pad, 256 valid, pad, pad-unused]
BAND = 8  # prefilter band rows
HALF = 4  # psum batch rows (4 rows x 512 f32 = 4 PSUM banks)

# module-level knobs for test harness
_trace = False
last_exec_ns = None
last_results = None

_PROGRAM_CACHE = {}


def _build_program(steps: int, repeat: int = 1) -> bass.Bass:
    nc = bacc.Bacc()
    xin = nc.declare_dram_parameter("xin", [128, 66, RS], BF, isOutput=False)
    m32 = nc.declare_dram_parameter(
        "m32", [steps, 128, GROW, COLS], BF, isOutput=False
    )
    wts = nc.declare_dram_parameter("wts", [128, 5, 128], BF, isOutput=False)
    be = nc.declare_dram_parameter("be", [128, 1], F32, isOutput=False)
    xout = nc.declare_dram_parameter("xout", [128, GROW, COLS], F32, isOutput=True)

    with tile.TileContext(nc) as tc:
        with (
            tc.tile_pool(name="state", bufs=1) as state,
            tc.tile_pool(name="singles", bufs=1) as singles,
            tc.tile_pool(name="sband", bufs=3) as sband,
            tc.tile_pool(name="taband", bufs=3) as taband,
            tc.tile_pool(name="tbband", bufs=3) as tbband,
            tc.tile_pool(name="psum", bufs=2, space="PSUM") as psum,
            tc.tile_pool(name="dband", bufs=3) as dband,
            tc.tile_pool(name="mband", bufs=3) as mband,
            tc.tile_pool(name="pmband", bufs=3) as pmband,
            tc.tile_pool(name="preband", bufs=3) as preband,
            tc.tile_pool(name="oband", bufs=3) as oband,
        ):
            XA = state.tile([128, 66, RS], BF, tag="XA")
            XB = state.tile([128, 66, RS], BF, tag="XB")
            w_sb = singles.tile([128, 5, 128], BF, tag="w")
            be_sb = singles.tile([128, 1], F32, tag="be")

            nc.sync.dma_start(out=w_sb[:], in_=wts[:])
            nc.sync.dma_start(out=be_sb[:], in_=be[:])

            X = [XA, XB]
            # taps: (weight idx, source kind, horizontal offset v)
            # sources: 0 = Xc (center), 1 = TA, 2 = TB
            taps = [
                (0, 0, 0),  # Wc        * x@0
                (1, 1, -1),  # Wdx/8    * tA@-1
                (1, 1, +1),  # Wdx/8    * tA@+1  (same weights as previous)
                (2, 1, 0),  # 2*Wdx/8   * tA@0
                (3, 2, +1),  # Wdy/8    * tB@+1
                (4, 2, -1),  # -Wdy/8   * tB@-1
            ]

            for _rep in range(repeat):
              nc.sync.dma_start(out=XA[:], in_=xin[:])
              nc.sync.dma_start(out=XB[:], in_=xin[:])
              for st in range(steps):
                Xc, Xn = X[st % 2], X[(st + 1) % 2]
                final = st == steps - 1
                for kb in range(ROWS // GROUPS // BAND):  # 8 bands of 8 rows
                    b0 = BAND * kb
                    S = sband.tile([128, BAND + 1, RS], BF, tag="S")
                    # s(r) = x(r) + x(r+1), r = b0-1 .. b0+7  (buffer rows b0..b0+8)
                    nc.vector.tensor_add(
                        S[:], Xc[:, b0 : b0 + 9, :], Xc[:, b0 + 1 : b0 + 10, :]
                    )
                    TA = taband.tile([128, BAND, RS], BF, tag="TA")
                    nc.vector.tensor_sub(TA[:], S[:, 1:9, :], S[:, 0:8, :])
                    TB = tbband.tile([128, BAND, RS], BF, tag="TB")
                    nc.vector.tensor_add(TB[:], S[:, 1:9, :], S[:, 0:8, :])

                    # one accumulation group per PSUM bank: 2 rows x 256 = 512
                    # fp32 = exactly one bank (start=True clears whole banks)
                    pt = psum.tile([128, BAND, COLS], F32, tag="pt")
                    for ti, (wi, src, v) in enumerate(taps):
                        first = ti == 0
                        last = ti == len(taps) - 1
                        for grp in range(BAND // 2):
                            q = 2 * grp
                            if src == 0:
                                rhs = Xc[:, b0 + q + 1 : b0 + q + 3, 2:258]
                            elif src == 1:
                                rhs = TA[:, q : q + 2, 2 + v : 258 + v]
                            else:
                                rhs = TB[:, q : q + 2, 2 + v : 258 + v]
                            nc.tensor.matmul(
                                pt[:, q : q + 2, :],
                                w_sb[:, wi, :],
                                rhs,
                                start=first,
                                stop=last,
                            )
                    # epilogue on the 8-row band
                    dsb = dband.tile([128, BAND, COLS], BF, tag="d")
                    nc.scalar.activation(
                        dsb[:],
                        pt[:],
                        mybir.ActivationFunctionType.Identity,
                        bias=be_sb[:, 0:1],
                        scale=1.0,
                    )
                    mb = mband.tile([128, BAND, COLS], BF, tag="m")
                    nc.sync.dma_start(out=mb[:], in_=m32[st, :, b0 : b0 + BAND, :])
                    pm = pmband.tile([128, BAND, COLS], BF, tag="pm")
                    nc.vector.tensor_mul(pm[:], dsb[:], mb[:])
                    pre = preband.tile([128, BAND, COLS], BF, tag="pre")
                    nc.gpsimd.tensor_add(
                        pre[:], pm[:], Xc[:, b0 + 1 : b0 + BAND + 1, 2:258]
                    )
                    if final:
                        ob = oband.tile([128, BAND, COLS], F32, tag="o")
                        nc.scalar.activation(
                            ob[:], pre[:], mybir.ActivationFunctionType.Sigmoid
                        )
                        nc.sync.dma_start(
                            out=xout[:, b0 : b0 + BAND, :], in_=ob[:]
                        )
                    else:
                        nc.scalar.activation(
                            Xn[:, b0 + 1 : b0 + BAND + 1, 2:258],
                            pre[:],
                            mybir.ActivationFunctionType.Sigmoid,
                        )
                if not final:
                    # restore pass-through channels 0..2 of each group
                    for g in range(GROUPS):
                        nc.sync.dma_start(
                            out=Xn[32 * g : 32 * g + 3, :, :],
                            in_=Xc[32 * g : 32 * g + 3, :, :],
                        )
                    # halo exchange between groups (bottom then top)
                    nc.sync.dma_start(out=Xn[0:96, 65, :], in_=Xn[32:128, 1, :])
                    nc.sync.dma_start(out=Xn[32:128, 0, :], in_=Xn[0:96, 64, :])
    nc.finalize()
    return nc


_repeat = 1


def _get_program(steps: int, repeat: int = 1) -> bass.Bass:
    key = (steps, repeat)
    if key not in _PROGRAM_CACHE:
        _PROGRAM_CACHE[key] = _build_program(steps, repeat)
    return _PROGRAM_CACHE[key]


def _compute_masks(steps: int, B: int, H: int, W: int) -> np.ndarray:
    """Reproduce the reference's jax.random fire masks exactly (threefry is
    backend-invariant)."""
    import jax

    cpu = jax.local_devices(backend="cpu")[0]
    out = np.empty((steps, B, H, W), np.bool_)
    with jax.default_device(cpu):
        key = jax.random.key(42)
        for st in range(steps):
            k = jax.random.fold_in(key, st)
            u = jax.random.uniform(k, (B, H, W, 1))
            out[st] = np.asarray(u)[..., 0] > 0.5
    return out


def kernel(x, fc0_w, fc0_b, fc1_w, steps):
    steps = int(steps)
    x = np.asarray(x, dtype=np.float32)
    fc0_w = np.asarray(fc0_w, dtype=np.float32)
    fc0_b = np.asarray(fc0_b, dtype=np.float32)
    fc1_w = np.asarray(fc1_w, dtype=np.float32)
    if steps <= 0:
        return x.copy()
    B, H, W, C = x.shape
    assert (B, H, W, C) == (N_CORES, ROWS, COLS, CH)

    # collapsed MLP weights
    W_eff = fc1_w.astype(np.float64) @ fc0_w.astype(np.float64)  # [32, 96]
    b_eff = fc1_w.astype(np.float64) @ fc0_b.astype(np.float64)  # [32]
    Wc = W_eff[:, :32]
    Wdx = W_eff[:, 32:64] / 8.0
    Wdy = W_eff[:, 64:96] / 8.0
    blocks = [Wc, Wdx, 2.0 * Wdx, Wdy, -Wdy]
    wts = np.zeros((128, 5, 128), np.float64)
    for t, Wt in enumerate(blocks):
        for g in range(GROUPS):
            wts[32 * g : 32 * g + 32, t, 32 * g : 32 * g + 32] = Wt.T
    wts = wts.astype(BF16)
    be = np.tile(b_eff.astype(np.float32), GROUPS).reshape(128, 1)

    masks = _compute_masks(steps, B, H, W)  # [steps, B, H, W] bool

    # build padded channel-partitioned input layout
    xt = x.transpose(0, 3, 1, 2)  # [B, C, H, W]
    xin = np.zeros((B, 128, 66, RS), np.float32)
    for g in range(GROUPS):
        p = slice(32 * g, 32 * g + 32)
        xin[:, p, 1:65, 2:258] = xt[:, :, 64 * g : 64 * g + 64, :]
        if g > 0:
            xin[:, p, 0, 2:258] = xt[:, :, 64 * g - 1, :]
        if g < GROUPS - 1:
            xin[:, p, 65, 2:258] = xt[:, :, 64 * g + 64, :]
    xin = xin.astype(BF16)

    # broadcast masks over channels into the partition layout
    m32 = np.empty((B, steps, 128, GROW, COLS), np.float32)
    mt = masks.transpose(1, 0, 2, 3)  # [B, steps, H, W]
    for g in range(GROUPS):
        m32[:, :, 32 * g : 32 * g + 32, :, :] = mt[
            :, :, None, 64 * g : 64 * g + 64, :
        ]
    m32 = m32.astype(BF16)

    nc = _get_program(steps, _repeat)
    in_maps = [
        {"xin": xin[b], "m32": m32[b], "wts": wts, "be": be} for b in range(B)
    ]
    res = run_bass_kernel_spmd(
        nc, in_maps, list(range(N_CORES)), trace=_trace
    )
    global last_exec_ns, last_results
    last_exec_ns = res.exec_time_ns
    last_results = res

    out = np.empty_like(x)
    for b in range(B):
        xo = res.results[b]["xout"]  # [128, 64, 256] f32
        for g in range(GROUPS):
            out[b, 64 * g : 64 * g + 64, :, :] = xo[
                32 * g : 32 * g + 32
            ].transpose(1, 2, 0)
    out[..., :3] = x[..., :3]
    return out


# revision 17
# speedup vs baseline: 28.4850x; 28.4850x over previous
"""Trainium2 Bass kernel for BasicNCA (noAddUp): 2 steps of
perceive(Sobel) -> linear MLP (collapsed, no activation between layers)
-> masked sigmoid update, data-parallel over batch across 8 NeuronCores.

Layout per core (batch element): x[256,256,32] -> SBUF [128, 66, 260] bf16
  partition p = 32*g + c   (g = row-group of 64 image rows, c = channel)
  buffer row br = image row (64g + br - 1); br=0/65 are halo rows
  col j: image col = j - 2 for j in [2, 258); j in {0,1,258,259} are zero pads

Math (exact collapse of the reference):
  W_eff = fc1_w @ fc0_w [32,96], b_eff = fc1_w @ fc0_b
  d = Wc*x + (Wdx/8)*(tA@-1 + 2 tA@0 + tA@+1) + (Wdy/8)*(tB@+1 - tB@-1) + b_eff
  where tA(r) = x(r+1)-x(r-1), tB(r) = x(r-1)+2x(r)+x(r+1)   (vertical)
  s(r) = x(r)+x(r+1);  tA = s(r)-s(r-1);  tB = s(r)+s(r-1)
  x_new = sigmoid(x + d*mask) on channels 3..31; channels 0..2 pass through.
"""

import sys

import numpy as np

for _p in ("/opt/trn_rl_repo", "/root/.axon_site/_ro/trn_rl_repo"):
    if _p not in sys.path:
        sys.path.append(_p)

import ml_dtypes  # noqa: E402

import concourse.bass as bass  # noqa: E402
import concourse.tile as tile  # noqa: E402
from concourse import bacc, mybir  # noqa: E402
from concourse.bass_utils import run_bass_kernel_spmd  # noqa: E402

BF16 = ml_dtypes.bfloat16
F32 = mybir.dt.float32
BF = mybir.dt.bfloat16

N_CORES = 8
ROWS = 256
COLS = 256
CH = 32
GROUPS = 4
GROW = ROWS // GROUPS  # 64 rows per group
RS = 260  # padded row stride: [pad, pad, 256 valid, pad, pad-unused]
BAND = 8  # prefilter band rows
HALF = 4  # psum batch rows (4 rows x 512 f32 = 4 PSUM banks)

# module-level knobs for test harness
_trace = False
last_exec_ns = None
last_results = None

_PROGRAM_CACHE = {}


def _build_program(steps: int, repeat: int = 1) -> bass.Bass:
    nc = bacc.Bacc()
    xin = nc.declare_dram_parameter("xin", [128, 66, RS], BF, isOutput=False)
    m32 = nc.declare_dram_parameter(
        "m32", [steps, 128, GROW, COLS], BF, isOutput=False
    )
    wts = nc.declare_dram_parameter("wts", [128, 5, 128], BF, isOutput=False)
    be = nc.declare_dram_parameter("be", [128, 1], F32, isOutput=False)
    xout = nc.declare_dram_parameter("xout", [128, GROW, COLS], F32, isOutput=True)

    with tile.TileContext(nc) as tc:
        with (
            tc.tile_pool(name="state", bufs=1) as state,
            tc.tile_pool(name="singles", bufs=1) as singles,
            tc.tile_pool(name="sband", bufs=3) as sband,
            tc.tile_pool(name="taband", bufs=3) as taband,
            tc.tile_pool(name="tbband", bufs=3) as tbband,
            tc.tile_pool(name="psum", bufs=2, space="PSUM") as psum,
            tc.tile_pool(name="dband", bufs=3) as dband,
            tc.tile_pool(name="mband", bufs=3) as mband,
            tc.tile_pool(name="pmband", bufs=3) as pmband,
            tc.tile_pool(name="preband", bufs=3) as preband,
            tc.tile_pool(name="oband", bufs=3) as oband,
        ):
            XA = state.tile([128, 66, RS], BF, tag="XA")
            XB = state.tile([128, 66, RS], BF, tag="XB")
            w_sb = singles.tile([128, 5, 128], BF, tag="w")
            be_sb = singles.tile([128, 1], F32, tag="be")

            nc.sync.dma_start(out=w_sb[:], in_=wts[:])
            nc.sync.dma_start(out=be_sb[:], in_=be[:])
            # XB only needs its pad columns/halo zeros before step 1 writes it;
            # ch0..2 rows and halos are refreshed by restore/halo DMAs.
            nc.sync.dma_start(out=XB[:], in_=xin[:])

            X = [XA, XB]
            # taps: (weight idx, source kind, horizontal offset v)
            # sources: 0 = Xc (center), 1 = TA, 2 = TB
            taps = [
                (0, 0, 0),  # Wc        * x@0
                (1, 1, -1),  # Wdx/8    * tA@-1
                (1, 1, +1),  # Wdx/8    * tA@+1  (same weights as previous)
                (2, 1, 0),  # 2*Wdx/8   * tA@0
                (3, 2, +1),  # Wdy/8    * tB@+1
                (4, 2, -1),  # -Wdy/8   * tB@-1
            ]

            def body(_i=None):
              nc.sync.dma_start(out=XA[:], in_=xin[:])
              for st in range(steps):
                Xc, Xn = X[st % 2], X[(st + 1) % 2]
                final = st == steps - 1
                for kb in range(ROWS // GROUPS // BAND):  # 8 bands of 8 rows
                    b0 = BAND * kb
                    S = sband.tile([128, BAND + 1, RS], BF, tag="S")
                    # s(r) = x(r) + x(r+1), r = b0-1 .. b0+7  (buffer rows b0..b0+8)
                    nc.vector.tensor_add(
                        S[:], Xc[:, b0 : b0 + 9, :], Xc[:, b0 + 1 : b0 + 10, :]
                    )
                    TA = taband.tile([128, BAND, RS], BF, tag="TA")
                    nc.vector.tensor_sub(TA[:], S[:, 1:9, :], S[:, 0:8, :])
                    TB = tbband.tile([128, BAND, RS], BF, tag="TB")
                    nc.vector.tensor_add(TB[:], S[:, 1:9, :], S[:, 0:8, :])

                    # one accumulation group per PSUM bank: 2 rows x 256 = 512
                    # fp32 = exactly one bank (start=True clears whole banks)
                    pt = psum.tile([128, BAND, COLS], F32, tag="pt")
                    for ti, (wi, src, v) in enumerate(taps):
                        first = ti == 0
                        last = ti == len(taps) - 1
                        for grp in range(BAND // 2):
                            q = 2 * grp
                            if src == 0:
                                rhs = Xc[:, b0 + q + 1 : b0 + q + 3, 2:258]
                            elif src == 1:
                                rhs = TA[:, q : q + 2, 2 + v : 258 + v]
                            else:
                                rhs = TB[:, q : q + 2, 2 + v : 258 + v]
                            nc.tensor.matmul(
                                pt[:, q : q + 2, :],
                                w_sb[:, wi, :],
                                rhs,
                                start=first,
                                stop=last,
                            )
                    # epilogue on the 8-row band
                    dsb = dband.tile([128, BAND, COLS], BF, tag="d")
                    nc.scalar.activation(
                        dsb[:],
                        pt[:],
                        mybir.ActivationFunctionType.Identity,
                        bias=be_sb[:, 0:1],
                        scale=1.0,
                    )
                    mb = mband.tile([128, BAND, COLS], BF, tag="m")
                    nc.sync.dma_start(out=mb[:], in_=m32[st, :, b0 : b0 + BAND, :])
                    pm = pmband.tile([128, BAND, COLS], BF, tag="pm")
                    nc.vector.tensor_mul(pm[:], dsb[:], mb[:])
                    pre = preband.tile([128, BAND, COLS], BF, tag="pre")
                    nc.gpsimd.tensor_add(
                        pre[:], pm[:], Xc[:, b0 + 1 : b0 + BAND + 1, 2:258]
                    )
                    if final:
                        ob = oband.tile([128, BAND, COLS], F32, tag="o")
                        nc.scalar.activation(
                            ob[:], pre[:], mybir.ActivationFunctionType.Sigmoid
                        )
                        nc.sync.dma_start(
                            out=xout[:, b0 : b0 + BAND, :], in_=ob[:]
                        )
                    else:
                        nc.scalar.activation(
                            Xn[:, b0 + 1 : b0 + BAND + 1, 2:258],
                            pre[:],
                            mybir.ActivationFunctionType.Sigmoid,
                        )
                if not final:
                    # restore pass-through channels 0..2 of each group
                    for g in range(GROUPS):
                        nc.sync.dma_start(
                            out=Xn[32 * g : 32 * g + 3, :, :],
                            in_=Xc[32 * g : 32 * g + 3, :, :],
                        )
                    # halo exchange between groups (bottom then top)
                    nc.sync.dma_start(out=Xn[0:96, 65, :], in_=Xn[32:128, 1, :])
                    nc.sync.dma_start(out=Xn[32:128, 0, :], in_=Xn[0:96, 64, :])

            if repeat == 1:
                body()
            else:
                with tc.For_i(0, repeat, 1) as _i:
                    body(_i)
    nc.finalize()
    return nc


_repeat = 1


def _get_program(steps: int, repeat: int = 1) -> bass.Bass:
    key = (steps, repeat)
    if key not in _PROGRAM_CACHE:
        _PROGRAM_CACHE[key] = _build_program(steps, repeat)
    return _PROGRAM_CACHE[key]


def _compute_masks(steps: int, B: int, H: int, W: int) -> np.ndarray:
    """Reproduce the reference's jax.random fire masks exactly (threefry is
    backend-invariant)."""
    import jax

    cpu = jax.local_devices(backend="cpu")[0]
    out = np.empty((steps, B, H, W), np.bool_)
    with jax.default_device(cpu):
        key = jax.random.key(42)
        for st in range(steps):
            k = jax.random.fold_in(key, st)
            u = jax.random.uniform(k, (B, H, W, 1))
            out[st] = np.asarray(u)[..., 0] > 0.5
    return out


def kernel(x, fc0_w, fc0_b, fc1_w, steps):
    steps = int(steps)
    x = np.asarray(x, dtype=np.float32)
    fc0_w = np.asarray(fc0_w, dtype=np.float32)
    fc0_b = np.asarray(fc0_b, dtype=np.float32)
    fc1_w = np.asarray(fc1_w, dtype=np.float32)
    if steps <= 0:
        return x.copy()
    B, H, W, C = x.shape
    assert (B, H, W, C) == (N_CORES, ROWS, COLS, CH)

    # collapsed MLP weights
    W_eff = fc1_w.astype(np.float64) @ fc0_w.astype(np.float64)  # [32, 96]
    b_eff = fc1_w.astype(np.float64) @ fc0_b.astype(np.float64)  # [32]
    Wc = W_eff[:, :32]
    Wdx = W_eff[:, 32:64] / 8.0
    Wdy = W_eff[:, 64:96] / 8.0
    blocks = [Wc, Wdx, 2.0 * Wdx, Wdy, -Wdy]
    wts = np.zeros((128, 5, 128), np.float64)
    for t, Wt in enumerate(blocks):
        for g in range(GROUPS):
            wts[32 * g : 32 * g + 32, t, 32 * g : 32 * g + 32] = Wt.T
    wts = wts.astype(BF16)
    be = np.tile(b_eff.astype(np.float32), GROUPS).reshape(128, 1)

    masks = _compute_masks(steps, B, H, W)  # [steps, B, H, W] bool

    # build padded channel-partitioned input layout
    xt = x.transpose(0, 3, 1, 2)  # [B, C, H, W]
    xin = np.zeros((B, 128, 66, RS), np.float32)
    for g in range(GROUPS):
        p = slice(32 * g, 32 * g + 32)
        xin[:, p, 1:65, 2:258] = xt[:, :, 64 * g : 64 * g + 64, :]
        if g > 0:
            xin[:, p, 0, 2:258] = xt[:, :, 64 * g - 1, :]
        if g < GROUPS - 1:
            xin[:, p, 65, 2:258] = xt[:, :, 64 * g + 64, :]
    xin = xin.astype(BF16)

    # broadcast masks over channels into the partition layout
    m32 = np.empty((B, steps, 128, GROW, COLS), np.float32)
    mt = masks.transpose(1, 0, 2, 3)  # [B, steps, H, W]
    for g in range(GROUPS):
        m32[:, :, 32 * g : 32 * g + 32, :, :] = mt[
            :, :, None, 64 * g : 64 * g + 64, :
        ]
    m32 = m32.astype(BF16)

    nc = _get_program(steps, _repeat)
    in_maps = [
        {"xin": xin[b], "m32": m32[b], "wts": wts, "be": be} for b in range(B)
    ]
    res = run_bass_kernel_spmd(
        nc, in_maps, list(range(N_CORES)), trace=_trace
    )
    global last_exec_ns, last_results
    last_exec_ns = res.exec_time_ns
    last_results = res

    out = np.empty_like(x)
    for b in range(B):
        xo = res.results[b]["xout"]  # [128, 64, 256] f32
        for g in range(GROUPS):
            out[b, 64 * g : 64 * g + 64, :, :] = xo[
                32 * g : 32 * g + 32
            ].transpose(1, 2, 0)
    out[..., :3] = x[..., :3]
    return out
